# revision 7
# baseline (speedup 1.0000x reference)
"""Dense soft-MoE (ExpertAdapter) Trainium2 Bass kernel.

Reference computation (per token t):
    r = softmax(x @ Wr + br)                      # [E]
    h_e = gelu(x @ W1[e] + b1[e])                 # [F]
    y_e = h_e @ W2[e] + b2[e]                     # [D]
    out = sum_e r[e] * y_e                        # [D]

Strategy: pure data-parallel over tokens across 8 NeuronCores (weights
replicated).  All layout work (x transpose, fp16 casts, weight
rearrangement) happens on the host; the device does only matmuls,
gelu, the router softmax, and the weighted combine:

  out[t,:] = rinv[t] * ( sum_e sum_f (gelu(hT)*exp_e)[f,t] W2[e][f,:]
                         + sum_e exp[e,t] b2[e,:] )
  where exp[e,t] = exp(logit[e,t] + br[e]),  rinv[t] = 1/sum_e exp[e,t]

The expert combine happens inside PSUM accumulation (router weights
folded into the GEMM2 lhs via an unnormalized-exp broadcast; the
softmax denominator is applied once at the end by DVE).

Per-rep structure:
  router prologue over all 8 big tiles (keeps ACT in the Exp table set
  for one contiguous stretch -> a single Exp<->Gelu table switch per
  rep instead of two per tile), then the GEMM main loop per big tile,
  software-pipelined one expert deep so PE never waits on the
  gelu->mult chain.  x segments are re-DMAed once per rep (tile t's
  segment refreshed right after its last GEMM1 read) so steady-state
  timing includes the input traffic.

Layouts (per 256-token big tile, 2 sub-tiles of 128 tokens):
  xT   [128d, (t, k, tok)] fp16   (host-transposed)
  hT   [128f, tok]         PSUM   (GEMM1: lhsT=W1 chunk, rhs=xT chunk)
  y    [128tok, 1024d]     PSUM   (GEMM2: lhsT=scaled hT chunk, rhs=W2)
"""

import numpy as np

import concourse.bacc as bacc
import concourse.mybir as mybir
import concourse.tile as tile
from concourse.bass_utils import run_bass_kernel_spmd

# Problem constants (hardcoded per spec nn_ExpertAdapter_13640816132847)
B, N, D, E, F = 4, 4096, 1024, 8, 512
N_CORES = 8
TOK_PER_CORE = (B * N) // N_CORES  # 2048

F16 = mybir.dt.float16
F32 = mybir.dt.float32

KD = D // 128     # 8 d-chunks
FC = F // 128     # 4 f-chunks
BT = 256          # tokens per big tile
NSUB = BT // 128  # 2 sub-tiles


def build_moe(n_tok=TOK_PER_CORE, repeat=1, sim_safe=False):
    """Build the per-core Bass program.  Inputs are host-prepared (see
    make_in_maps): fp16 weights in PE-ready layouts and the fp16
    pre-transposed x shard.

    sim_safe=True replaces Gelu (not implemented in CoreSim) with Identity
    so the dataflow can be validated in simulation; hardware uses Gelu."""
    assert n_tok % BT == 0
    nbt = n_tok // BT
    gelu_fn = (mybir.ActivationFunctionType.Identity if sim_safe
               else mybir.ActivationFunctionType.Gelu)

    nc = bacc.Bacc("TRN2", target_bir_lowering=False, debug=False)

    xT_d = nc.dram_tensor("xT", [128, nbt, KD, BT], F16, kind="ExternalInput").ap()
    w1_d = nc.dram_tensor("W1t", [128, E * KD * F], F16, kind="ExternalInput").ap()
    w2_d = nc.dram_tensor("W2t", [128, E * FC * D], F16, kind="ExternalInput").ap()
    wr_d = nc.dram_tensor("Wrt", [128, KD * E], F16, kind="ExternalInput").ap()
    b1_d = nc.dram_tensor("b1c", [128, E * FC], F32, kind="ExternalInput").ap()
    b2_d = nc.dram_tensor("b2h", [E, D], F16, kind="ExternalInput").ap()
    br_d = nc.dram_tensor("brc", [E, 1], F32, kind="ExternalInput").ap()
    out_d = nc.dram_tensor("out", [n_tok, D], F32, kind="ExternalOutput").ap()

    with tile.TileContext(nc) as tc:
        with (
            tc.tile_pool(name="consts", bufs=1) as consts,
            tc.tile_pool(name="wres", bufs=1) as wres,
            tc.tile_pool(name="routp", bufs=2) as routp,
            tc.tile_pool(name="wesp", bufs=2) as wesp,
            tc.tile_pool(name="hgp", bufs=2) as hgp,
            tc.tile_pool(name="hsp", bufs=2) as hsp,
            tc.tile_pool(name="outp", bufs=2) as outp,
            tc.tile_pool(name="ps_y", bufs=2, space="PSUM") as ps_y,
            tc.tile_pool(name="ps_h", bufs=1, space="PSUM") as ps_h,
            tc.tile_pool(name="ps_r", bufs=2, space="PSUM") as ps_r,
        ):
            # ---- constants / weights (once, outside the rep loop) ----
            ones8 = consts.tile([E, 1], F16)
            nc.gpsimd.memset(ones8[:], 1.0)
            wr_sb = consts.tile([128, KD * E], F16)
            nc.sync.dma_start(wr_sb[:], wr_d[:])
            b1_sb = consts.tile([128, E * FC], F32)
            nc.sync.dma_start(b1_sb[:], b1_d[:])
            b2_sb = consts.tile([E, D], F16)
            nc.sync.dma_start(b2_sb[:], b2_d[:])
            br_sb = consts.tile([E, 1], F32)
            nc.sync.dma_start(br_sb[:, 0:1], br_d[:, 0:1])

            w1_sb = wres.tile([128, E * KD * F], F16)
            w2_sb = wres.tile([128, E * FC * D], F16, name="w2_sb")
            for e in range(E):
                nc.sync.dma_start(w1_sb[:, e * KD * F:(e + 1) * KD * F],
                                  w1_d[:, e * KD * F:(e + 1) * KD * F])
                nc.sync.dma_start(w2_sb[:, e * FC * D:(e + 1) * FC * D],
                                  w2_d[:, e * FC * D:(e + 1) * FC * D])
            xt_sb = wres.tile([128, nbt, KD, BT], F16, name="xt_sb")
            for t in range(nbt):
                nc.sync.dma_start(xt_sb[:, t, :, :], xT_d[:, t, :, :])

            for rep in range(repeat):
                # ---- router prologue, all tiles.  exp(l) is computed as
                # (1+tanh(l/2))/(1-tanh(l/2)): Tanh shares the ACT table set
                # with Gelu, so the whole kernel needs zero table switches
                # (Exp lives in a different set; switching costs ~2.7us).
                exps, rinvs = [], []
                for t in range(nbt):
                    rt = ps_r.tile([128, 512], F32, tag="rt")
                    for k in range(KD):
                        nc.tensor.matmul(
                            rt[0:E, 0:BT],
                            wr_sb[:, k * E:(k + 1) * E],
                            xt_sb[:, t, k, :],
                            start=(k == 0), stop=(k == KD - 1))
                    th = routp.tile([E, BT], F32, tag="th", bufs=2)
                    nc.scalar.activation(th[:], rt[0:E, 0:BT],
                                         mybir.ActivationFunctionType.Tanh,
                                         bias=br_sb[:], scale=0.5)
                    num = routp.tile([E, BT], F32, tag="num", bufs=2)
                    nc.vector.tensor_scalar_add(num[:], th[:], 1.0)
                    den = routp.tile([E, BT], F32, tag="den", bufs=2)
                    nc.vector.tensor_scalar(den[:], th[:], -1.0, 1.0,
                                            op0=mybir.AluOpType.mult,
                                            op1=mybir.AluOpType.add)
                    denr = routp.tile([E, BT], F32, tag="denr", bufs=2)
                    nc.vector.reciprocal(denr[:], den[:])
                    exp_h = routp.tile([E, BT], F16, tag="exp", bufs=nbt)
                    nc.vector.tensor_mul(exp_h[:], num[:], denr[:])
                    nc.tensor.matmul(rt[0:1, 256:256 + BT], ones8[:], exp_h[:],
                                     start=True, stop=True)
                    sums_sb = routp.tile([1, BT], F32, tag="sums", bufs=2)
                    nc.vector.tensor_copy(sums_sb[0:1, :], rt[0:1, 256:256 + BT])
                    rv = []
                    for s in range(NSUB):
                        s_col = routp.tile([128, 1], F32, tag="scol", bufs=2)
                        nc.sync.dma_start(s_col[:, 0:1],
                                          sums_sb[0:1, s * 128:(s + 1) * 128])
                        rinv = routp.tile([128, 1], F32, tag="rinv",
                                          bufs=NSUB * nbt)
                        nc.vector.reciprocal(rinv[:], s_col[:])
                        rv.append(rinv)
                    exps.append(exp_h)
                    rinvs.append(rv)

                def emit_wes(t):
                    """Broadcast tile t's unnormalized router weights to all
                    128 partitions (gpsimd, off the critical path)."""
                    exp_row = routp.tile([1, E * BT], F16, tag="expr", bufs=2)
                    nc.sync.dma_start(exp_row[0:1, :], exps[t][:])
                    wt = []
                    for e in range(E):
                        we = wesp.tile([128, BT], F16, tag="we", name="we",
                                       bufs=2 * E)
                        nc.gpsimd.partition_broadcast(
                            we[:], exp_row[0:1, e * BT:(e + 1) * BT])
                        wt.append(we)
                    return wt

                wes = emit_wes(0)

                # ---- main loop ----
                for t in range(nbt):
                    # next tile's router broadcasts first so gpsimd works ahead
                    wes_next = emit_wes(t + 1) if t + 1 < nbt else None

                    ys = [ps_y.tile([128, D], F32, tag="y", name=f"y{s}")
                          for s in range(NSUB)]

                    def emit_gemm2(e, hs_e):
                        for fc in range(FC):
                            for s in range(NSUB):
                                for dh in range(2):
                                    nc.tensor.matmul(
                                        ys[s][:, dh * 512:(dh + 1) * 512],
                                        hs_e[:, fc * BT + s * 128:
                                             fc * BT + (s + 1) * 128],
                                        w2_sb[:, (e * FC + fc) * D + dh * 512:
                                              (e * FC + fc) * D + (dh + 1) * 512],
                                        start=False,
                                        stop=(e == E - 1 and fc == FC - 1))

                    # hT PSUM: one 2-bank tile per big tile, one quarter-slot
                    # per fc group, so GEMM1(e+1,fc)'s start waits on
                    # gelu(e,fc) -- 4 groups (~3.5us) of slack instead of 1
                    hps = ps_h.tile([128, FC * BT], F32, tag="h", name="hps")

                    # expert loop, software-pipelined: GEMM2(e) emitted after
                    # GEMM1(e+1) so PE never waits on gelu->mult producing hs
                    prev = None
                    for e in range(E):
                        hs = hsp.tile([128, FC * BT], F16, tag="hs")
                        for fc in range(FC):
                            hreg = hps[:, fc * BT:fc * BT + BT]
                            for k in range(KD):
                                nc.tensor.matmul(
                                    hreg,
                                    w1_sb[:, (e * KD + k) * F + fc * 128:
                                          (e * KD + k) * F + (fc + 1) * 128],
                                    xt_sb[:, t, k, :],
                                    start=(k == 0), stop=(k == KD - 1))
                            hg = hgp.tile([128, BT], F16, tag="hg", bufs=4)
                            nc.scalar.activation(
                                hg[:], hreg, gelu_fn,
                                bias=b1_sb[:, e * FC + fc: e * FC + fc + 1],
                                scale=1.0)
                            nc.vector.tensor_mul(
                                hs[:, fc * BT:(fc + 1) * BT], hg[:], wes[e])
                        if e == 1:
                            # b2 (weighted by exp) opens each ys accumulation
                            # group; placed here so it executes after GEMM1(e1)
                            # (PE is in-order) giving DVE time to drain the
                            # previous tile's ys with the rinv scale.
                            for s in range(NSUB):
                                for dh in range(2):
                                    nc.tensor.matmul(
                                        ys[s][:, dh * 512:(dh + 1) * 512],
                                        exps[t][:, s * 128:(s + 1) * 128],
                                        b2_sb[:, dh * 512:(dh + 1) * 512],
                                        start=True, stop=False)
                        if prev is not None:
                            emit_gemm2(*prev)
                        prev = (e, hs)
                    emit_gemm2(*prev)

                    # refresh this tile's x segment for the next rep (keeps
                    # per-rep HBM input traffic in the steady-state timing)
                    if rep + 1 < repeat:
                        nc.sync.dma_start(xt_sb[:, t, :, :], xT_d[:, t, :, :])

                    # scale by 1/sums on DVE, store
                    for s in range(NSUB):
                        o_sb = outp.tile([128, D], F32, tag="osb")
                        nc.vector.tensor_scalar_mul(o_sb[:], ys[s][:],
                                                    rinvs[t][s][:])
                        nc.sync.dma_start(
                            out_d[t * BT + s * 128: t * BT + (s + 1) * 128, :],
                            o_sb[:])
                    wes = wes_next

    nc.compile()
    return nc


def make_in_maps(x, W1, b1, W2, b2, Wr, br):
    """Host-side layout prep: shard x over cores, pre-transpose/cast to the
    PE-ready fp16 layouts declared in build_moe."""
    x = np.ascontiguousarray(np.asarray(x, dtype=np.float32)).reshape(B * N, D)
    W1 = np.asarray(W1, dtype=np.float32)
    b1 = np.asarray(b1, dtype=np.float32)
    W2 = np.asarray(W2, dtype=np.float32)
    b2 = np.asarray(b2, dtype=np.float32)
    Wr = np.asarray(Wr, dtype=np.float32)
    br = np.asarray(br, dtype=np.float32)

    nbt = TOK_PER_CORE // BT
    # [p, (e k) f] = W1[e, k*128+p, f]
    w1t = np.ascontiguousarray(
        W1.reshape(E, KD, 128, F).transpose(2, 0, 1, 3).reshape(128, E * KD * F)
    ).astype(np.float16)
    # [p, (e c) d] = W2[e, c*128+p, d]
    w2t = np.ascontiguousarray(
        W2.reshape(E, FC, 128, D).transpose(2, 0, 1, 3).reshape(128, E * FC * D)
    ).astype(np.float16)
    # [p, k e] = Wr[k*128+p, e]
    wrt = np.ascontiguousarray(
        Wr.reshape(KD, 128, E).transpose(1, 0, 2).reshape(128, KD * E)
    ).astype(np.float16)
    # [p, e c] = b1[e, c*128+p]
    b1c = np.ascontiguousarray(
        b1.reshape(E, FC, 128).transpose(2, 0, 1).reshape(128, E * FC))
    b2h = b2.astype(np.float16)
    # tanh-based exp needs br/2 (the ACT computes tanh(l*0.5 + br*0.5))
    brc = np.ascontiguousarray(0.5 * br.reshape(E, 1))

    in_maps = []
    for c in range(N_CORES):
        xs = x[c * TOK_PER_CORE:(c + 1) * TOK_PER_CORE]  # [n_tok, D]
        # [p, t, k, tok] = xs[t*BT+tok, k*128+p]
        xT = np.ascontiguousarray(
            xs.reshape(nbt, BT, KD, 128).transpose(3, 0, 2, 1)
        ).astype(np.float16)
        in_maps.append({"xT": xT, "W1t": w1t, "W2t": w2t, "Wrt": wrt,
                        "b1c": b1c, "b2h": b2h, "brc": brc})
    return in_maps


_NC_CACHE = {}


def _get_nc(n_tok=TOK_PER_CORE, repeat=1):
    key = (n_tok, repeat)
    if key not in _NC_CACHE:
        _NC_CACHE[key] = build_moe(n_tok, repeat)
    return _NC_CACHE[key]


def kernel(x, W1, b1, W2, b2, Wr, br):
    in_maps = make_in_maps(x, W1, b1, W2, b2, Wr, br)
    nc = _get_nc()
    try:
        res = run_bass_kernel_spmd(nc, in_maps, core_ids=list(range(N_CORES)))
    except Exception:
        # A previously-wedged NeuronCore clears with a core reset on retry.
        import os
        os.environ.setdefault("NEURON_RT_RESET_CORES", "1")
        res = run_bass_kernel_spmd(nc, in_maps, core_ids=list(range(N_CORES)))
    out = np.concatenate([r["out"] for r in res.results], axis=0)
    return out.reshape(B, N, D).astype(np.float32)


# revision 10
# speedup vs baseline: 1.0490x; 1.0490x over previous
"""Dense soft-MoE (ExpertAdapter) Trainium2 Bass kernel.

Reference computation (per token t):
    r = softmax(x @ Wr + br)                      # [E]
    h_e = gelu(x @ W1[e] + b1[e])                 # [F]
    y_e = h_e @ W2[e] + b2[e]                     # [D]
    out = sum_e r[e] * y_e                        # [D]

Strategy: pure data-parallel over tokens across 8 NeuronCores (weights
replicated).  All layout work (x transpose, fp16 casts, weight
rearrangement) happens on the host; the device does only matmuls,
gelu, the router softmax, and the weighted combine:

  out[t,:] = rinv[t] * ( sum_e sum_f (gelu(hT)*exp_e)[f,t] W2[e][f,:]
                         + sum_e exp[e,t] b2[e,:] )
  where exp[e,t] = exp(logit[e,t] + br[e]),  rinv[t] = 1/sum_e exp[e,t]

The expert combine happens inside PSUM accumulation (router weights
folded into the GEMM2 lhs via an unnormalized-exp broadcast; the
softmax denominator is applied once at the end by DVE).

Per-rep structure:
  router prologue over all 8 big tiles (keeps ACT in the Exp table set
  for one contiguous stretch -> a single Exp<->Gelu table switch per
  rep instead of two per tile), then the GEMM main loop per big tile,
  software-pipelined one expert deep so PE never waits on the
  gelu->mult chain.  x segments are re-DMAed once per rep (tile t's
  segment refreshed right after its last GEMM1 read) so steady-state
  timing includes the input traffic.

Layouts (per 256-token big tile, 2 sub-tiles of 128 tokens):
  xT   [128d, (t, k, tok)] fp16   (host-transposed)
  hT   [128f, tok]         PSUM   (GEMM1: lhsT=W1 chunk, rhs=xT chunk)
  y    [128tok, 1024d]     PSUM   (GEMM2: lhsT=scaled hT chunk, rhs=W2)
"""

import numpy as np

import concourse.bacc as bacc
import concourse.mybir as mybir
import concourse.tile as tile
from concourse.bass_utils import run_bass_kernel_spmd

# Problem constants (hardcoded per spec nn_ExpertAdapter_13640816132847)
B, N, D, E, F = 4, 4096, 1024, 8, 512
N_CORES = 8
TOK_PER_CORE = (B * N) // N_CORES  # 2048

F16 = mybir.dt.float16
F32 = mybir.dt.float32

KD = D // 128     # 8 d-chunks
FC = F // 128     # 4 f-chunks
BT = 256          # tokens per big tile
NSUB = BT // 128  # 2 sub-tiles


def build_moe(n_tok=TOK_PER_CORE, repeat=1, sim_safe=False):
    """Build the per-core Bass program.  Inputs are host-prepared (see
    make_in_maps): fp16 weights in PE-ready layouts and the fp16
    pre-transposed x shard.

    sim_safe=True replaces Gelu (not implemented in CoreSim) with Identity
    so the dataflow can be validated in simulation; hardware uses Gelu."""
    assert n_tok % BT == 0
    nbt = n_tok // BT
    gelu_fn = (mybir.ActivationFunctionType.Identity if sim_safe
               else mybir.ActivationFunctionType.Gelu)

    nc = bacc.Bacc("TRN2", target_bir_lowering=False, debug=False)

    xT_d = nc.dram_tensor("xT", [128, nbt, KD, BT], F16, kind="ExternalInput").ap()
    w1_d = nc.dram_tensor("W1t", [128, E * KD * F], F16, kind="ExternalInput").ap()
    w2_d = nc.dram_tensor("W2t", [128, E * FC * D], F16, kind="ExternalInput").ap()
    wr_d = nc.dram_tensor("Wrt", [128, KD * E], F16, kind="ExternalInput").ap()
    b1_d = nc.dram_tensor("b1c", [128, E * FC], F32, kind="ExternalInput").ap()
    b2_d = nc.dram_tensor("b2h", [E, D], F16, kind="ExternalInput").ap()
    br_d = nc.dram_tensor("brc", [E, 1], F32, kind="ExternalInput").ap()
    out_d = nc.dram_tensor("out", [n_tok, D], F32, kind="ExternalOutput").ap()

    with tile.TileContext(nc) as tc:
        with (
            tc.tile_pool(name="consts", bufs=1) as consts,
            tc.tile_pool(name="wres", bufs=1) as wres,
            tc.tile_pool(name="routp", bufs=2) as routp,
            tc.tile_pool(name="wesp", bufs=2) as wesp,
            tc.tile_pool(name="hgp", bufs=2) as hgp,
            tc.tile_pool(name="hsp", bufs=2) as hsp,
            tc.tile_pool(name="outp", bufs=2) as outp,
            tc.tile_pool(name="ps_y", bufs=2, space="PSUM") as ps_y,
            tc.tile_pool(name="ps_h", bufs=1, space="PSUM") as ps_h,
            tc.tile_pool(name="ps_r", bufs=1, space="PSUM") as ps_r,
        ):
            # ---- constants / weights (once, outside the rep loop) ----
            ones8 = consts.tile([E, 1], F16)
            nc.gpsimd.memset(ones8[:], 1.0)
            wr_sb = consts.tile([128, KD * E], F16)
            nc.sync.dma_start(wr_sb[:], wr_d[:])
            b1_sb = consts.tile([128, E * FC], F32)
            nc.sync.dma_start(b1_sb[:], b1_d[:])
            b2_sb = consts.tile([E, D], F16)
            nc.sync.dma_start(b2_sb[:], b2_d[:])
            br_sb = consts.tile([E, 1], F32)
            nc.sync.dma_start(br_sb[:, 0:1], br_d[:, 0:1])

            w1_sb = wres.tile([128, E * KD * F], F16)
            w2_sb = wres.tile([128, E * FC * D], F16, name="w2_sb")
            for e in range(E):
                nc.sync.dma_start(w1_sb[:, e * KD * F:(e + 1) * KD * F],
                                  w1_d[:, e * KD * F:(e + 1) * KD * F])
                nc.sync.dma_start(w2_sb[:, e * FC * D:(e + 1) * FC * D],
                                  w2_d[:, e * FC * D:(e + 1) * FC * D])
            xt_sb = wres.tile([128, nbt, KD, BT], F16, name="xt_sb")
            for t in range(nbt):
                nc.sync.dma_start(xt_sb[:, t, :, :], xT_d[:, t, :, :])

            for rep in range(repeat):
                # ---- router prologue, all tiles.  exp(l) is computed as
                # (1+tanh(l/2))/(1-tanh(l/2)): Tanh shares the ACT table set
                # with Gelu, so the whole kernel needs zero table switches
                # (Exp lives in a different set; switching costs ~2.7us).
                exps, rinvs = [], []
                for t in range(nbt):
                    rt = ps_r.tile([128, 512], F32, tag="rt")
                    for k in range(KD):
                        nc.tensor.matmul(
                            rt[0:E, 0:BT],
                            wr_sb[:, k * E:(k + 1) * E],
                            xt_sb[:, t, k, :],
                            start=(k == 0), stop=(k == KD - 1))
                    th = routp.tile([E, BT], F32, tag="th", bufs=2)
                    nc.scalar.activation(th[:], rt[0:E, 0:BT],
                                         mybir.ActivationFunctionType.Tanh,
                                         bias=br_sb[:], scale=0.5)
                    num = routp.tile([E, BT], F32, tag="num", bufs=2)
                    nc.vector.tensor_scalar_add(num[:], th[:], 1.0)
                    den = routp.tile([E, BT], F32, tag="den", bufs=2)
                    nc.vector.tensor_scalar(den[:], th[:], -1.0, 1.0,
                                            op0=mybir.AluOpType.mult,
                                            op1=mybir.AluOpType.add)
                    denr = routp.tile([E, BT], F32, tag="denr", bufs=2)
                    nc.vector.reciprocal(denr[:], den[:])
                    exp_h = routp.tile([E, BT], F16, tag="exp", bufs=nbt)
                    nc.vector.tensor_mul(exp_h[:], num[:], denr[:])
                    nc.tensor.matmul(rt[0:1, 256:256 + BT], ones8[:], exp_h[:],
                                     start=True, stop=True)
                    sums_sb = routp.tile([1, BT], F32, tag="sums", bufs=2)
                    nc.vector.tensor_copy(sums_sb[0:1, :], rt[0:1, 256:256 + BT])
                    rv = []
                    for s in range(NSUB):
                        s_col = routp.tile([128, 1], F32, tag="scol", bufs=2)
                        nc.sync.dma_start(s_col[:, 0:1],
                                          sums_sb[0:1, s * 128:(s + 1) * 128])
                        rinv = routp.tile([128, 1], F32, tag="rinv",
                                          bufs=NSUB * nbt)
                        nc.vector.reciprocal(rinv[:], s_col[:])
                        rv.append(rinv)
                    exps.append(exp_h)
                    rinvs.append(rv)

                def emit_wes(t):
                    """Broadcast tile t's unnormalized router weights to all
                    128 partitions (gpsimd, off the critical path)."""
                    exp_row = routp.tile([1, E * BT], F16, tag="expr", bufs=2)
                    nc.sync.dma_start(exp_row[0:1, :], exps[t][:])
                    wt = []
                    for e in range(E):
                        we = wesp.tile([128, BT], F16, tag="we", name="we",
                                       bufs=2 * E)
                        nc.gpsimd.partition_broadcast(
                            we[:], exp_row[0:1, e * BT:(e + 1) * BT])
                        wt.append(we)
                    return wt

                wes = emit_wes(0)

                # hT PSUM: a single 3-bank region holding a 6-deep ring of
                # [128f, BT] fc-group slots.  GEMM1 group g's start then only
                # waits on gelu(g-6) -- ~2 experts of slack -- instead of the
                # just-in-time 4-slot rotation that left ~50ns of margin.
                hps = ps_h.tile([128, 6 * BT], F32, tag="h", name="hps")
                NSLOT = 6
                gslot = 0

                # ---- main loop ----
                for t in range(nbt):
                    # next tile's router broadcasts first so gpsimd works ahead
                    wes_next = emit_wes(t + 1) if t + 1 < nbt else None

                    ys = [ps_y.tile([128, D], F32, tag="y", name=f"y{s}")
                          for s in range(NSUB)]

                    def emit_gemm2(e, hs_e):
                        for fc in range(FC):
                            for s in range(NSUB):
                                for dh in range(2):
                                    nc.tensor.matmul(
                                        ys[s][:, dh * 512:(dh + 1) * 512],
                                        hs_e[:, fc * BT + s * 128:
                                             fc * BT + (s + 1) * 128],
                                        w2_sb[:, (e * FC + fc) * D + dh * 512:
                                              (e * FC + fc) * D + (dh + 1) * 512],
                                        start=False,
                                        stop=(e == E - 1 and fc == FC - 1))

                    # expert loop, software-pipelined: GEMM2(e) emitted after
                    # GEMM1(e+1) so PE never waits on gelu->mult producing hs
                    prev = None
                    for e in range(E):
                        hs = hsp.tile([128, FC * BT], F16, tag="hs")
                        for fc in range(FC):
                            hreg = hps[:, gslot * BT:gslot * BT + BT]
                            gslot = (gslot + 1) % NSLOT
                            for k in range(KD):
                                nc.tensor.matmul(
                                    hreg,
                                    w1_sb[:, (e * KD + k) * F + fc * 128:
                                          (e * KD + k) * F + (fc + 1) * 128],
                                    xt_sb[:, t, k, :],
                                    start=(k == 0), stop=(k == KD - 1))
                            hg = hgp.tile([128, BT], F16, tag="hg", bufs=4)
                            nc.scalar.activation(
                                hg[:], hreg, gelu_fn,
                                bias=b1_sb[:, e * FC + fc: e * FC + fc + 1],
                                scale=1.0)
                            nc.vector.tensor_mul(
                                hs[:, fc * BT:(fc + 1) * BT], hg[:], wes[e])
                        if e == 1:
                            # b2 (weighted by exp) opens each ys accumulation
                            # group; placed here so it executes after GEMM1(e1)
                            # (PE is in-order) giving DVE time to drain the
                            # previous tile's ys with the rinv scale.
                            for s in range(NSUB):
                                for dh in range(2):
                                    nc.tensor.matmul(
                                        ys[s][:, dh * 512:(dh + 1) * 512],
                                        exps[t][:, s * 128:(s + 1) * 128],
                                        b2_sb[:, dh * 512:(dh + 1) * 512],
                                        start=True, stop=False)
                        if prev is not None:
                            emit_gemm2(*prev)
                        prev = (e, hs)
                    emit_gemm2(*prev)

                    # refresh this tile's x segment for the next rep (keeps
                    # per-rep HBM input traffic in the steady-state timing)
                    if rep + 1 < repeat:
                        nc.sync.dma_start(xt_sb[:, t, :, :], xT_d[:, t, :, :])

                    # scale by 1/sums on DVE, store
                    for s in range(NSUB):
                        o_sb = outp.tile([128, D], F32, tag="osb")
                        nc.vector.tensor_scalar_mul(o_sb[:], ys[s][:],
                                                    rinvs[t][s][:])
                        nc.sync.dma_start(
                            out_d[t * BT + s * 128: t * BT + (s + 1) * 128, :],
                            o_sb[:])
                    wes = wes_next

    nc.compile()
    return nc


def make_in_maps(x, W1, b1, W2, b2, Wr, br):
    """Host-side layout prep: shard x over cores, pre-transpose/cast to the
    PE-ready fp16 layouts declared in build_moe."""
    x = np.ascontiguousarray(np.asarray(x, dtype=np.float32)).reshape(B * N, D)
    W1 = np.asarray(W1, dtype=np.float32)
    b1 = np.asarray(b1, dtype=np.float32)
    W2 = np.asarray(W2, dtype=np.float32)
    b2 = np.asarray(b2, dtype=np.float32)
    Wr = np.asarray(Wr, dtype=np.float32)
    br = np.asarray(br, dtype=np.float32)

    nbt = TOK_PER_CORE // BT
    # [p, (e k) f] = W1[e, k*128+p, f]
    w1t = np.ascontiguousarray(
        W1.reshape(E, KD, 128, F).transpose(2, 0, 1, 3).reshape(128, E * KD * F)
    ).astype(np.float16)
    # [p, (e c) d] = W2[e, c*128+p, d]
    w2t = np.ascontiguousarray(
        W2.reshape(E, FC, 128, D).transpose(2, 0, 1, 3).reshape(128, E * FC * D)
    ).astype(np.float16)
    # [p, k e] = Wr[k*128+p, e]
    wrt = np.ascontiguousarray(
        Wr.reshape(KD, 128, E).transpose(1, 0, 2).reshape(128, KD * E)
    ).astype(np.float16)
    # [p, e c] = b1[e, c*128+p]
    b1c = np.ascontiguousarray(
        b1.reshape(E, FC, 128).transpose(2, 0, 1).reshape(128, E * FC))
    b2h = b2.astype(np.float16)
    # tanh-based exp needs br/2 (the ACT computes tanh(l*0.5 + br*0.5))
    brc = np.ascontiguousarray(0.5 * br.reshape(E, 1))

    in_maps = []
    for c in range(N_CORES):
        xs = x[c * TOK_PER_CORE:(c + 1) * TOK_PER_CORE]  # [n_tok, D]
        # [p, t, k, tok] = xs[t*BT+tok, k*128+p]
        xT = np.ascontiguousarray(
            xs.reshape(nbt, BT, KD, 128).transpose(3, 0, 2, 1)
        ).astype(np.float16)
        in_maps.append({"xT": xT, "W1t": w1t, "W2t": w2t, "Wrt": wrt,
                        "b1c": b1c, "b2h": b2h, "brc": brc})
    return in_maps


_NC_CACHE = {}


def _get_nc(n_tok=TOK_PER_CORE, repeat=1):
    key = (n_tok, repeat)
    if key not in _NC_CACHE:
        _NC_CACHE[key] = build_moe(n_tok, repeat)
    return _NC_CACHE[key]


def kernel(x, W1, b1, W2, b2, Wr, br):
    in_maps = make_in_maps(x, W1, b1, W2, b2, Wr, br)
    nc = _get_nc()
    try:
        res = run_bass_kernel_spmd(nc, in_maps, core_ids=list(range(N_CORES)))
    except Exception:
        # A previously-wedged NeuronCore clears with a core reset on retry.
        import os
        os.environ.setdefault("NEURON_RT_RESET_CORES", "1")
        res = run_bass_kernel_spmd(nc, in_maps, core_ids=list(range(N_CORES)))
    out = np.concatenate([r["out"] for r in res.results], axis=0)
    return out.reshape(B, N, D).astype(np.float32)
